# revision 8
# baseline (speedup 1.0000x reference)
"""Trainium2 Bass kernel for histogram_binning (FDS feature smoothing).

Computation (per sample n, feature d):
    b      = clip(int(clip(label_n,0,1)*50), 0, 49)
    factor = clip(v2p[b,d]/max(v1p[b,d],1e-12), 0.1, 10)
    out    = (feat - m1[b,d])*sqrt(factor) + m2[b,d]      (where v1p>0, else feat)

Strategy: data-parallel over samples across 8 cores. On each core:
  - fold the affine into out = feat*scale[b] + bias[b]; tables [50,128]
    computed once on device, split hi/lo in bf16 (error ~2^-18 rel).
  - per-sample gather via one-hot matmul on TensorE:
      * bucket idx per sample precomputed in a batched pass (exact floor via
        convert + compare, robust to any convert rounding mode), staged to
        DRAM, re-read 1KB/tile at partition 0.
      * K=1 matmul broadcasts idx across 100 partitions (PSUM),
        ScalarE: a = Abs(idx - b), oh = Relu(1 - a)  -> exact one-hot (bf16)
      * one K=100, N=256 matmul per 128-sample block gathers
        [scale_hi;scale_lo] and [bias_hi;bias_lo] (columns 0:128 scale,
        128:256 bias) into PSUM.
  - VectorE: out = feat*scale_g + bias_g (two fp32 tensor_tensor ops).
  - tiles of 512 samples; feat DMA'd 2KB-contiguous per partition
    (partition p holds samples 4p..4p+3 of the tile); the idx vector is
    permuted once so one-hot columns line up with that layout.
"""

import os
import sys

import numpy as np

for _p in ("/opt/trn_rl_repo",):
    if _p not in sys.path:
        sys.path.insert(0, _p)

import concourse.bacc as bacc
import concourse.bass as bass
import concourse.tile as tile
from concourse import mybir
from concourse.bass_utils import run_bass_kernel_spmd

F32 = mybir.dt.float32
BF16 = mybir.dt.bfloat16
I32 = mybir.dt.int32
OP = mybir.AluOpType
AF = mybir.ActivationFunctionType

N_CORES = 8
N_FULL = 1_000_000
D = 128
B = 50
TILE_S = 512          # samples per tile
KB = 4                # consecutive samples per partition (TILE_S = 128*KB)
GRP = 5               # tiles per DMA group (SWDGE emission amortization)

# Padded sizes: per-core sample count must be a multiple of GRP*TILE_S, and
# the label staging layout needs n_slots (= padded_tiles) to be a multiple
# of 128.
PER_CORE = 125_440    # 49 groups * 5 tiles * 512
N_TILES = PER_CORE // TILE_S


def _slot_layout(n_tiles: int) -> int:
    """number of 128-slot groups (H) in the idx staging layout"""
    return (n_tiles + 127) // 128


def build_nc(per_core: int = PER_CORE):
    """Build the single-core Bass program (SPMD across 8 cores)."""
    assert per_core % (GRP * TILE_S) == 0
    n_tiles = per_core // TILE_S
    H = _slot_layout(n_tiles)
    n_slots = H * 128
    lab_pad = n_slots * TILE_S

    # Bacc (not plain Bass): its compile() legalizes Tile's multi-sem-wait
    # sync_info into event semaphores — walrus codegen supports only one
    # embedded wait per ISA instruction.
    nc = bacc.Bacc("TRN2", target_bir_lowering=False, debug=False)
    feat = nc.dram_tensor("features", [per_core, D], F32, kind="ExternalInput")
    lab = nc.dram_tensor("labels", [lab_pad], F32, kind="ExternalInput")
    m1 = nc.dram_tensor("m1", [B, D], F32, kind="ExternalInput")
    v1 = nc.dram_tensor("v1", [B, D], F32, kind="ExternalInput")
    m2 = nc.dram_tensor("m2", [B, D], F32, kind="ExternalInput")
    v2 = nc.dram_tensor("v2", [B, D], F32, kind="ExternalInput")
    out = nc.dram_tensor("out", [per_core, D], F32, kind="ExternalOutput")

    with tile.TileContext(nc) as tc:
        _build_body(nc, tc, feat, lab, m1, v1, m2, v2, out, n_tiles, H)
    nc.compile()
    return nc


def _build_body(nc, tc, feat, lab, m1, v1, m2, v2, out, n_tiles, H):
    import contextlib

    ctx = contextlib.ExitStack()
    with ctx:
        const = ctx.enter_context(tc.tile_pool(name="const", bufs=1))
        prep = ctx.enter_context(tc.tile_pool(name="prep", bufs=1))
        dramp = ctx.enter_context(tc.tile_pool(name="dramp", bufs=1, space="DRAM"))
        feat_pool = ctx.enter_context(tc.tile_pool(name="featp", bufs=4))
        out_pool = ctx.enter_context(tc.tile_pool(name="outp", bufs=4))
        oh_pool = ctx.enter_context(tc.tile_pool(name="ohp", bufs=3))
        sq_pool = ctx.enter_context(tc.tile_pool(name="sqp", bufs=3))
        idxt_pool = ctx.enter_context(tc.tile_pool(name="idxtp", bufs=4))
        pz_pool = ctx.enter_context(tc.tile_pool(name="pzp", bufs=2, space="PSUM"))
        pg_pool = ctx.enter_context(tc.tile_pool(name="pgp", bufs=2, space="PSUM"))

        # ---------------- one-time: tables ----------------
        m1_sb = const.tile([B, D], F32, tag="m1")
        v1_sb = const.tile([B, D], F32, tag="v1")
        m2_sb = const.tile([B, D], F32, tag="m2")
        v2_sb = const.tile([B, D], F32, tag="v2")
        nc.sync.dma_start(out=m1_sb, in_=m1[:, :])
        nc.sync.dma_start(out=v1_sb, in_=v1[:, :])
        nc.sync.dma_start(out=m2_sb, in_=m2[:, :])
        nc.sync.dma_start(out=v2_sb, in_=v2[:, :])

        t_a = const.tile([B, D], F32, tag="ta")
        t_b = const.tile([B, D], F32, tag="tb")
        t_c = const.tile([B, D], F32, tag="tc")
        mask = const.tile([B, D], F32, tag="mask")
        scale_f = const.tile([B, D], F32, tag="scalef")
        bias_f = const.tile([B, D], F32, tag="biasf")

        # factor = clip(max(v2,0) / max(v1, 1e-12), 0.1, 10)
        nc.vector.tensor_scalar(t_a, v2_sb, 0.0, None, OP.max)          # v2p
        nc.vector.tensor_scalar(t_b, v1_sb, 1e-12, None, OP.max)        # den
        nc.vector.reciprocal(t_c, t_b)                                   # 1/den
        nc.vector.tensor_tensor(t_a, t_a, t_c, OP.mult)                  # ratio
        nc.vector.tensor_scalar(t_a, t_a, 0.1, 10.0, OP.max, OP.min)     # clip
        # scale0 = sqrt(factor); one Newton step: s1 = 0.5*(s0 + factor/s0)
        nc.scalar.activation(t_b, t_a, AF.Sqrt)                          # s0
        nc.vector.reciprocal(t_c, t_b)                                   # 1/s0
        nc.vector.tensor_tensor(t_c, t_a, t_c, OP.mult)                  # factor/s0
        nc.vector.tensor_tensor(t_b, t_b, t_c, OP.add)
        nc.vector.tensor_scalar(t_b, t_b, 0.5, None, OP.mult)            # s1
        # mask = v1 > 0 ; scale = 1 + mask*(s1-1) ; bias = mask*(m2 - m1*scale)
        nc.vector.tensor_scalar(mask, v1_sb, 0.0, None, OP.is_gt)
        nc.vector.tensor_scalar(t_b, t_b, 1.0, None, OP.subtract)
        nc.vector.tensor_tensor(t_b, mask, t_b, OP.mult)
        nc.vector.tensor_scalar(scale_f, t_b, 1.0, None, OP.add)
        nc.vector.tensor_tensor(t_a, m1_sb, scale_f, OP.mult)
        nc.vector.tensor_tensor(t_a, m2_sb, t_a, OP.subtract)
        nc.vector.tensor_tensor(bias_f, mask, t_a, OP.mult)

        # hi/lo bf16 split, packed into tabs2 [100, 256] bf16:
        #   rows 0:50 hi, rows 50:100 lo; cols 0:128 scale, cols 128:256 bias
        tabs2 = const.tile([2 * B, 2 * D], BF16, tag="tabs2")
        lo_bf = const.tile([B, D], BF16, tag="lobf")
        for src, col in ((scale_f, 0), (bias_f, D)):
            nc.vector.tensor_copy(tabs2[0:B, col:col + D], src)          # hi (bf16)
            nc.vector.tensor_copy(t_c, tabs2[0:B, col:col + D])          # hi -> f32
            nc.vector.tensor_tensor(t_c, src, t_c, OP.subtract)          # err
            nc.vector.tensor_copy(lo_bf, t_c)                            # lo (bf16)
            nc.gpsimd.dma_start(out=tabs2[B:2 * B, col:col + D], in_=lo_bf[:, :])

        # ones row for the K=1 broadcast matmul; -(p%50) bias column for ACT
        ones_sb = const.tile([1, 2 * B], BF16, tag="ones")
        nc.vector.memset(ones_sb, 1.0)
        ni32 = const.tile([2 * B, 1], I32, tag="ni32")
        nc.gpsimd.iota(ni32, [[1, 1]], channel_multiplier=1)
        nif = const.tile([2 * B, 1], F32, tag="nif")
        nc.vector.tensor_copy(nif, ni32)                                 # p as f32
        nwrap = const.tile([2 * B, 1], F32, tag="nwrap")
        nc.vector.tensor_scalar(nwrap, nif, float(B), float(B), OP.is_ge, OP.mult)
        neg_iota = const.tile([2 * B, 1], F32, tag="negiota")
        nc.vector.tensor_tensor(neg_iota, nwrap, nif, OP.subtract)       # -(p%50)

        # ---------------- one-time: bucket indices ----------------
        # staging layout: slot t = h*128 + p  ->  partition p, block h
        lab_t = prep.tile([128, H, TILE_S], F32, tag="lab")
        lab_r = lab.rearrange("(h p j) -> p h j", p=128, j=TILE_S)
        nc.sync.dma_start(out=lab_t, in_=lab_r)

        u_t = prep.tile([128, H, TILE_S], F32, tag="u")
        r_t = prep.tile([128, H, TILE_S], I32, tag="r")
        f_t = prep.tile([128, H, TILE_S], F32, tag="f")
        d_t = prep.tile([128, H, TILE_S], F32, tag="d")
        g_t = prep.tile([128, H, TILE_S], F32, tag="g")
        idx_bf = prep.tile([128, H, TILE_S], BF16, tag="idxbf")

        nc.vector.tensor_scalar(u_t, lab_t, 0.0, 50.0, OP.max, OP.mult)  # u
        nc.vector.tensor_copy(r_t, u_t)                                  # int(u)
        nc.vector.tensor_copy(f_t, r_t)                                  # back to f32
        nc.vector.tensor_tensor(d_t, f_t, u_t, OP.is_gt)                 # f > u
        nc.vector.tensor_tensor(g_t, f_t, d_t, OP.subtract)              # floor(u)
        # permute within each tile: idx_bf[p,h,k*128+m] = g[p,h,4m+k]
        g_v = g_t.rearrange("p h (m k) -> p h m k", k=KB)
        idx_v = idx_bf.rearrange("p h (k m) -> p h m k", k=KB)
        nc.vector.tensor_copy(idx_v, g_v)

        idx_dram = dramp.tile([H * 128, TILE_S], BF16, tag="idxdram")
        nc.gpsimd.dma_start(
            out=idx_dram.rearrange("(h p) j -> p h j", p=128), in_=idx_bf
        )

        # ---------------- main loop ----------------
        # SWDGE (gpsimd) DMAs: the only DGE path where Tile's multi-wait
        # sync_info compiles (HWDGE DIRECT2D supports a single wait).
        # Batch GRP tiles per DMA to amortize Q7 descriptor emission.
        n_groups = n_tiles // GRP
        feat5 = feat.rearrange("(G g p k) d -> G p g k d", g=GRP, p=128, k=KB)
        out5 = out.rearrange("(G g p k) d -> G p g k d", g=GRP, p=128, k=KB)

        for G in range(n_groups):
            idx_g = idxt_pool.tile([1, GRP * TILE_S], BF16, tag="idxg")
            nc.gpsimd.dma_start(out=idx_g, in_=idx_dram[G * GRP:(G + 1) * GRP, :])

            feat_g = feat_pool.tile([128, GRP, KB, D], F32, tag="feat")
            nc.gpsimd.dma_start(out=feat_g, in_=feat5[G])
            out_g = out_pool.tile([128, GRP, KB, D], F32, tag="out")

            for g in range(GRP):
                psum_z = pz_pool.tile([2 * B, TILE_S], F32, tag="pz")
                nc.tensor.matmul(
                    psum_z, ones_sb,
                    idx_g[0:1, g * TILE_S:(g + 1) * TILE_S],
                    start=True, stop=True,
                )

                sq_t = sq_pool.tile([2 * B, TILE_S], F32, tag="sq")
                nc.scalar.activation(sq_t, psum_z, AF.Abs, bias=neg_iota[:, 0:1])
                oh_t = oh_pool.tile([2 * B, TILE_S], BF16, tag="oh")
                nc.scalar.activation(oh_t, sq_t, AF.Relu, bias=1.0, scale=-1.0)

                psum_g = pg_pool.tile([128, KB, 2 * D], F32, tag="pg")
                for q in range(KB):
                    nc.tensor.matmul(
                        psum_g[:, q, :],
                        oh_t[:, q * 128:(q + 1) * 128],
                        tabs2,
                        start=True,
                        stop=True,
                    )

                nc.vector.tensor_tensor(
                    out_g[:, g], feat_g[:, g], psum_g[:, :, 0:D], OP.mult
                )
                nc.vector.tensor_tensor(
                    out_g[:, g], out_g[:, g], psum_g[:, :, D:2 * D], OP.add
                )

            nc.gpsimd.dma_start(out=out5[G], in_=out_g)


_NC_CACHE = {}


def _get_nc(per_core: int = PER_CORE):
    if per_core not in _NC_CACHE:
        _NC_CACHE[per_core] = build_nc(per_core)
    return _NC_CACHE[per_core]


def make_in_maps(features, labels, m1, v1, m2, v2, per_core=PER_CORE, n_cores=N_CORES):
    """Shard + pad the full inputs into per-core input maps."""
    features = np.ascontiguousarray(np.asarray(features, dtype=np.float32))
    labels = np.ascontiguousarray(np.asarray(labels, dtype=np.float32))
    n = features.shape[0]
    base = n // n_cores
    assert n % n_cores == 0
    n_tiles = per_core // TILE_S
    lab_pad = _slot_layout(n_tiles) * 128 * TILE_S

    m1 = np.asarray(m1, dtype=np.float32)
    v1 = np.asarray(v1, dtype=np.float32)
    m2 = np.asarray(m2, dtype=np.float32)
    v2 = np.asarray(v2, dtype=np.float32)

    in_maps = []
    for c in range(n_cores):
        fshard = np.zeros((per_core, features.shape[1]), dtype=np.float32)
        fshard[:base] = features[c * base:(c + 1) * base]
        lshard = np.zeros((lab_pad,), dtype=np.float32)
        lshard[:base] = labels[c * base:(c + 1) * base]
        in_maps.append(
            {"features": fshard, "labels": lshard,
             "m1": m1, "v1": v1, "m2": m2, "v2": v2}
        )
    return in_maps


def run_on_hw(inputs, trace=False, **kw):
    nc = _get_nc()
    in_maps = make_in_maps(**inputs)
    res = run_bass_kernel_spmd(nc, in_maps, list(range(N_CORES)), trace=trace, **kw)
    base = N_FULL // N_CORES
    full = np.concatenate(
        [res.results[c]["out"][:base] for c in range(N_CORES)], axis=0
    )
    return full, res


def kernel(**inputs) -> np.ndarray:
    out, _ = run_on_hw(inputs, trace=False)
    return out
